# revision 32
# baseline (speedup 1.0000x reference)
"""Trainium2 Bass kernel for CEN patch expert (im2col + patch-norm + 122-512-128-1 MLP).

Strategy (8 NeuronCores, data-parallel over batch B=32 -> 4 images/core):
  - Patch stats (mean/std over the 121 pixels of each 11x11 patch) computed
    separably from the image: vertical band-matmul + horizontal sliding-sum
    (log-shift adds on DVE), giving S = sum(p), Ssq = sum(p^2) per position.
  - Normalization folded into the first matmul:
        h1_pre = Wp @ (p * inv) - rowsum(Wp) * (mean * inv) + (W1[:,0] + b1)
    rhs rows = [p*inv (121); mean*inv (1); std*inv = 1 (1)]  (K = 123)
    lhsT rows = [Wp.T; -rowsum; W1[:,0]+b1]
  - im2col via overlapping-AP DMAs (11 per image) into a [123, 9410] SBUF
    buffer (row 121 = mean, row 122 = std so std*inv = 1 supplies the ones
    row); inv broadcast per tile from a DRAM scratch row (step-0 AP).
  - All MLP matmuls in float32r (full PE rate, ~1e-4 rel err, even-N ISA rule
    satisfied by padding positions 9409 -> 9410).
  - tanh/sigmoid on ScalarE (the bottleneck engine, ~82% busy): tanh1 over
    2-bank PSUM granules, MM2/MM3 reuse the first granule after tanh1 drains
    it ("greuse": 2 PSUM allocs/tile over 4 slots), sigmoid batched x4 tiles
    through SBUF staging, zero DMAs issued from the ScalarE ring in steady
    state. Stats computed per image so image 0 starts early; image 0's
    im2col fans across all 3 DGE rings to parallelize issue cost.
"""

import numpy as np

import concourse.bacc as bacc
import concourse.bass as bass
import concourse.tile as tile
import concourse.mybir as mybir
from concourse.bass_utils import run_bass_kernel_spmd

N_CORES = 8
B = 32
H = 107
PATCH = 11
R = 97          # output rows/cols
L = R * R       # 9409 positions per image
K = PATCH * PATCH  # 121
IPC = B // N_CORES  # images per core = 4
LP = L + 1      # padded positions (even last tile for fp32r matmul ISA)
NT = 512        # positions per tile
NTILES = (LP + NT - 1) // NT  # 19 (18x512 + 194)
GROUP = 4       # tiles per MM3/sigmoid group (col-tiling)

F32 = mybir.dt.float32
F32R = mybir.dt.float32r
Tanh = mybir.ActivationFunctionType.Tanh
Sigmoid = mybir.ActivationFunctionType.Sigmoid
Sqrt = mybir.ActivationFunctionType.Sqrt


def build(psum_mode="greuse", sig_stage=True, bufs_bc=6, bufs_rhs=6,
          bufs_h1=3, bufs_h2=3, im2col_eng="sync", group=GROUP):
    nc = bacc.Bacc("TRN2", target_bir_lowering=False, debug=False,
                   num_devices=N_CORES)
    x4 = nc.dram_tensor("x4", (IPC, H, H), F32, kind="ExternalInput")
    w1e = nc.dram_tensor("w1e", (123, 512), F32R, kind="ExternalInput")
    w2t = nc.dram_tensor("w2t", (128, 512), F32R, kind="ExternalInput")
    w3t = nc.dram_tensor("w3t", (128, 1), F32R, kind="ExternalInput")
    b2c = nc.dram_tensor("b2c", (128, 1), F32, kind="ExternalInput")
    b3c = nc.dram_tensor("b3c", (1, 1), F32, kind="ExternalInput")
    av = nc.dram_tensor("av", (H, R), F32, kind="ExternalInput")
    y4 = nc.dram_tensor("y4", (IPC, L), F32, kind="ExternalOutput")
    invflat = nc.dram_tensor("invflat", (IPC, LP), F32, kind="Internal")

    xt = x4.ap().tensor
    invt_d = invflat.ap().tensor

    with tile.TileContext(nc) as tc:
        with (
            tc.tile_pool(name="wp", bufs=1) as wp,
            tc.tile_pool(name="stat", bufs=1) as st,
            tc.tile_pool(name="pim", bufs=2) as pim,
            tc.tile_pool(name="bcp", bufs=bufs_bc) as bcp,
            tc.tile_pool(name="rhp", bufs=bufs_rhs) as rhp,
            tc.tile_pool(name="h1p", bufs=bufs_h1) as h1p,
            tc.tile_pool(name="h2p", bufs=bufs_h2) as h2p,
            tc.tile_pool(name="outp", bufs=2) as outp,
            tc.tile_pool(name="srp", bufs=2) as srp,
            tc.tile_pool(name="pg",
                         bufs=(4 if psum_mode in ("granule", "greuse") else 2),
                         space="PSUM") as pg,
        ):
            # ---- weights / consts ----
            w1s = wp.tile([123, 512], F32R, tag="w1s")
            nc.sync.dma_start(out=w1s, in_=w1e.ap()[:, :])
            w2s = wp.tile([128, 512], F32R, tag="w2s")
            nc.sync.dma_start(out=w2s, in_=w2t.ap()[:, :])
            w3s = wp.tile([128, 1], F32R, tag="w3s")
            nc.sync.dma_start(out=w3s, in_=w3t.ap()[:, :])
            b2s = wp.tile([128, 1], F32, tag="b2s")
            nc.sync.dma_start(out=b2s, in_=b2c.ap()[:, :])
            b3s = wp.tile([128, 1], F32, tag="b3s")
            nc.sync.dma_start(
                out=b3s,
                in_=bass.AP(tensor=b3c.ap().tensor, offset=0,
                            ap=[[0, 128], [1, 1]]))
            avs = wp.tile([H, R], F32, tag="avs")
            nc.sync.dma_start(out=avs, in_=av.ap()[:, :])
            onesf = wp.tile([1, NT], F32, tag="onesf")
            nc.vector.memset(onesf, 1.0)
            # pad column (position L) of invflat for all images, constant 1.0
            nc.sync.dma_start(
                out=bass.AP(tensor=invt_d, offset=L, ap=[[LP, IPC], [1, 1]]),
                in_=bass.AP(tensor=onesf.tensor, offset=onesf.offset,
                            ap=[onesf.ap[0], [0, IPC], [1, 1]]))

            # im2col machinery; image 0 emitted BEFORE phase A so its DMAs
            # overlap the stats chain (no dependency between them).
            im2eng = nc.gpsimd if im2col_eng == "gpsimd" else nc.sync

            def emit_im2col(img, spread=False):
                pimg = pim.tile([123, LP], F32, tag="pimg", name=f"pimg{img}")
                nc.vector.memset(pimg[:, L:LP], 0.0)
                # spread=True: fan the 11 DMAs over all three DGE rings so
                # their serial issue cost (~1.7us each) parallelizes -- used
                # for image 0 where nothing else needs the rings yet.
                engs = ([nc.sync, nc.scalar, nc.gpsimd] if spread
                        else [im2eng])
                for kh in range(PATCH):
                    engs[kh % len(engs)].dma_start(
                        out=pimg[kh * PATCH:(kh + 1) * PATCH, 0:L]
                            .rearrange("p (i j) -> p i j", i=R),
                        in_=bass.AP(tensor=xt, offset=img * H * H + kh * H,
                                    ap=[[1, PATCH], [H, R], [1, R]]))
                return pimg

            # xall[r, img, c] = x4[img, r, c] -- phase A input, load FIRST
            xall = st.tile([H, IPC, H], F32, tag="xall")
            nc.sync.dma_start(
                out=xall,
                in_=bass.AP(tensor=xt, offset=0,
                            ap=[[H, H], [H * H, IPC], [1, H]]))

            pimg0 = emit_im2col(0, spread=True)

            # ---- Phase A: per-position patch stats for all 4 images ----
            xsq = st.tile([H, IPC, H], F32, tag="xsq")
            nc.vector.tensor_mul(xsq, xall, xall)

            # vertical band sums: V[i, img, c] = sum_kh x[i+kh, img, c]
            W4 = IPC * H  # 428
            GW = 1024 if psum_mode in ("granule", "greuse") else 2048
            vtile = pg.tile([128, GW], F32, tag="g")
            for img in range(IPC):
                nc.tensor.matmul(vtile[0:R, img * H:(img + 1) * H],
                                 lhsT=avs, rhs=xall[:, img, :],
                                 start=True, stop=True)
                nc.tensor.matmul(vtile[0:R, GW // 2 + img * H:GW // 2 + (img + 1) * H],
                                 lhsT=avs, rhs=xsq[:, img, :],
                                 start=True, stop=True)
            vv = st.tile([R, 2 * W4], F32, tag="vv")  # [97, 856]: V | Vsq
            nc.vector.tensor_copy(vv[:, 0:W4], vtile[0:R, 0:W4])
            nc.vector.tensor_copy(vv[:, W4:2 * W4], vtile[0:R, GW // 2:GW // 2 + W4])

            # horizontal sliding sum of 11 via log-shift adds, PER IMAGE so
            # image 0's stats (and its first tiles) start ~4x earlier; the
            # other images' stats overlap image 0's compute. Each image
            # processes its V and Vsq segments together via a 2-segment
            # strided view ([97, 2, w]).
            meant = st.tile([R, IPC, R], F32, tag="meant")
            stdt = st.tile([R, IPC, R], F32, tag="stdt")
            invs = st.tile([R, IPC, R], F32, tag="invs")
            for img in range(IPC):
                def vseg(o, w):
                    return bass.AP(tensor=vv.tensor,
                                   offset=vv.offset + img * H + o,
                                   ap=[vv.ap[0], [W4, 2], [1, w]])
                w2v = st.tile([R, 2, H - 1], F32, tag="w2v")
                nc.vector.tensor_add(w2v, vseg(0, H - 1), vseg(1, H - 1))
                w4v = st.tile([R, 2, H - 3], F32, tag="w4v")
                nc.vector.tensor_add(w4v, w2v[:, :, 0:H - 3], w2v[:, :, 2:H - 1])
                w8v = st.tile([R, 2, H - 7], F32, tag="w8v")
                nc.vector.tensor_add(w8v, w4v[:, :, 0:H - 7], w4v[:, :, 4:H - 3])
                tvv = st.tile([R, 2, R], F32, tag="tvv")
                nc.vector.tensor_add(tvv, w8v[:, :, 0:R], w2v[:, :, 8:8 + R])
                sv = st.tile([R, 2, R], F32, tag="sv")  # [:,0,:]=S, [:,1,:]=Ssq
                nc.vector.tensor_add(sv, tvv, vseg(10, R))

                t1 = st.tile([R, R], F32, tag="t1")
                nc.vector.tensor_mul(t1, sv[:, 0, :], sv[:, 0, :])
                u = st.tile([R, R], F32, tag="u")
                # u = Ssq - S^2/121
                nc.vector.scalar_tensor_tensor(
                    out=u, in0=t1, scalar=-1.0 / K, in1=sv[:, 1, :],
                    op0=mybir.AluOpType.mult, op1=mybir.AluOpType.add)
                # std = sqrt(u / 120)
                nc.scalar.activation(out=stdt[:, img, :], in_=u, func=Sqrt,
                                     bias=0.0, scale=1.0 / (K - 1))
                nc.vector.reciprocal(invs[:, img, :], stdt[:, img, :])
                nc.vector.tensor_scalar_mul(meant[:, img, :], sv[:, 0, :],
                                            1.0 / K)
            # ---- Phase B: im2col + MLP per image ----
            for img in range(IPC):
                nc.sync.dma_start(
                    out=bass.AP(tensor=invt_d, offset=img * LP,
                                ap=[[R, R], [1, R]]),
                    in_=invs[:, img, :])
                pimg = pimg0 if img == 0 else emit_im2col(img)
                # (mean/std row DMAs are emitted right below; they execute
                # during the previous image's tail alongside im2col)
                # mean row (121) and std row (122; std*inv = 1 in rhs)
                nc.sync.dma_start(
                    out=pimg[121:122, 0:L].rearrange("p (i j) -> p i j", i=R),
                    in_=meant[:, img, :])
                nc.sync.dma_start(
                    out=pimg[122:123, 0:L].rearrange("p (i j) -> p i j", i=R),
                    in_=stdt[:, img, :])

                ngroups = (NTILES + group - 1) // group
                for g in range(ngroups):
                    t0 = g * group
                    gsz = min(group, NTILES - t0)
                    if sig_stage:
                        srow = srp.tile([1, group * NT], F32, tag="srow")
                    else:
                        outs_pt = outp.tile([1, group * NT], F32, tag="outs")
                    scols = 0
                    sc_list = []
                    for j in range(gsz):
                        t = t0 + j
                        n0 = t * NT
                        nt = min(NT, LP - n0)
                        bc = bcp.tile([123, NT], F32, tag="bc")
                        nc.sync.dma_start(
                            out=bc[:, 0:nt],
                            in_=bass.AP(tensor=invt_d, offset=img * LP + n0,
                                        ap=[[0, 123], [1, nt]]))
                        rhs = rhp.tile([123, NT], F32R, tag="rhs")
                        nc.vector.tensor_mul(rhs[:, 0:nt],
                                             pimg[:, n0:n0 + nt],
                                             bc[:, 0:nt])
                        h1 = h1p.tile([128, 4, NT], F32R, tag="h1")
                        if psum_mode == "reuse":
                            gt = pg.tile([128, 2048], F32, tag="g")
                            for c in range(4):
                                nc.tensor.matmul(
                                    gt[:, c * NT:c * NT + nt],
                                    lhsT=w1s[:, c * 128:(c + 1) * 128],
                                    rhs=rhs[:, 0:nt],
                                    start=True, stop=True)
                            nc.scalar.activation(
                                out=h1[:, :, 0:nt],
                                in_=gt.rearrange("p (c n) -> p c n", c=4)[:, :, 0:nt],
                                func=Tanh)
                            s23 = gt  # reuse the same 4-bank tile
                        elif psum_mode == "merged":
                            gt = pg.tile([128, 2048], F32, tag="g")
                            for c in range(4):
                                nc.tensor.matmul(
                                    gt[:, c * NT:c * NT + nt],
                                    lhsT=w1s[:, c * 128:(c + 1) * 128],
                                    rhs=rhs[:, 0:nt],
                                    start=True, stop=True)
                            nc.scalar.activation(
                                out=h1[:, :, 0:nt],
                                in_=gt.rearrange("p (c n) -> p c n", c=4)[:, :, 0:nt],
                                func=Tanh)
                        else:
                            gt0 = None
                            for gg in range(2):
                                gt = pg.tile([128, 1024], F32, tag="g")
                                if gg == 0:
                                    gt0 = gt
                                for c in range(2):
                                    mc = gg * 2 + c
                                    nc.tensor.matmul(
                                        gt[:, c * NT:c * NT + nt],
                                        lhsT=w1s[:, mc * 128:(mc + 1) * 128],
                                        rhs=rhs[:, 0:nt],
                                        start=True, stop=True)
                                nc.scalar.activation(
                                    out=h1[:, 2 * gg:2 * gg + 2, 0:nt],
                                    in_=gt.rearrange("p (c n) -> p c n", c=2)[:, :, 0:nt],
                                    func=Tanh)
                        if psum_mode == "greuse":
                            s23 = gt0  # reuse first granule after tanh1 read
                        elif psum_mode != "reuse":
                            s23 = pg.tile(
                                [128, 2048 if psum_mode == "merged" else 1024],
                                F32, tag="g")
                        for c in range(4):
                            nc.tensor.matmul(
                                s23[:, 0:nt],
                                lhsT=w2s[:, c * 128:(c + 1) * 128],
                                rhs=h1[:, c, 0:nt],
                                start=(c == 0), stop=(c == 3))
                        h2 = h2p.tile([128, NT], F32R, tag="h2")
                        nc.scalar.activation(out=h2[:, 0:nt],
                                             in_=s23[:, 0:nt],
                                             func=Tanh, bias=b2s[:, 0:1])
                        p3off = 1024 if psum_mode in ("merged", "reuse") else 512
                        nc.tensor.matmul(s23[0:1, p3off:p3off + nt],
                                         lhsT=w3s, rhs=h2[:, 0:nt],
                                         start=True, stop=True)
                        if sig_stage:
                            nc.vector.tensor_copy(
                                srow[0:1, scols:scols + nt],
                                s23[0:1, p3off:p3off + nt])
                        else:
                            nc.scalar.activation(
                                out=outs_pt[0:1, scols:scols + nt],
                                in_=s23[0:1, p3off:p3off + nt],
                                func=Sigmoid, bias=b3s[0:1, 0:1])
                        sc_list.append((n0, nt, min(nt, L - n0)))
                        scols += nt
                    base = t0 * NT
                    nout = sum(o for (_, _, o) in sc_list)
                    if sig_stage:
                        outs = outp.tile([1, group * NT], F32, tag="outs")
                        nc.scalar.activation(out=outs[0:1, 0:scols],
                                             in_=srow[0:1, 0:scols],
                                             func=Sigmoid, bias=b3s[0:1, 0:1])
                    else:
                        outs = outs_pt
                    nc.gpsimd.dma_start(
                        out=bass.AP(tensor=y4.ap().tensor,
                                    offset=img * L + base,
                                    ap=[[1, 1], [1, nout]]),
                        in_=outs[0:1, 0:nout])
    nc.compile()
    return nc


def prep_inputs(x, W1, b1, W2, b2, W3, b3):
    x = np.asarray(x, dtype=np.float32)
    W1 = np.asarray(W1, dtype=np.float32)
    b1 = np.asarray(b1, dtype=np.float32)
    W2 = np.asarray(W2, dtype=np.float32)
    b2 = np.asarray(b2, dtype=np.float32)
    W3 = np.asarray(W3, dtype=np.float32)
    b3 = np.asarray(b3, dtype=np.float32)

    Wp = W1[:, 1:]  # (512, 121)
    w1e = np.concatenate(
        [Wp.T, -Wp.sum(axis=1)[None, :], (W1[:, 0] + b1)[None, :]],
        axis=0).astype(np.float32)  # (123, 512)
    w2t = np.concatenate(
        [W2[:, c * 128:(c + 1) * 128].T for c in range(4)],
        axis=1).astype(np.float32)  # (128, 512)
    w3t = W3.T.astype(np.float32).copy()  # (128, 1)
    b2c = b2[:, None].astype(np.float32).copy()
    b3c = b3.reshape(1, 1).astype(np.float32).copy()
    av = np.zeros((H, R), dtype=np.float32)
    for i in range(R):
        av[i:i + PATCH, i] = 1.0

    shared = {"w1e": w1e, "w2t": w2t, "w3t": w3t,
              "b2c": b2c, "b3c": b3c, "av": av}
    in_maps = []
    for c in range(N_CORES):
        m = dict(shared)
        m["x4"] = np.ascontiguousarray(x[c * IPC:(c + 1) * IPC, 0])
        in_maps.append(m)
    return in_maps


_CACHE = {}


def kernel(x, W1, b1, W2, b2, W3, b3):
    nc = _CACHE.get("nc")
    if nc is None:
        nc = build(**_CACHE.get("build_kwargs", {}))
        _CACHE["nc"] = nc
    in_maps = prep_inputs(x, W1, b1, W2, b2, W3, b3)
    res = run_bass_kernel_spmd(nc, in_maps, core_ids=list(range(N_CORES)))
    y = np.stack([res.results[c]["y4"] for c in range(N_CORES)])  # (8,4,L)
    return y.reshape(B, 1, R, R).astype(np.float32)


if __name__ == "__main__":
    rng = np.random.default_rng(0)
    inputs = {
        "x": rng.standard_normal((B, 1, H, H), dtype=np.float32),
        "W1": (rng.standard_normal((512, 122)) * 0.05).astype(np.float32),
        "b1": (rng.standard_normal((512,)) * 0.05).astype(np.float32),
        "W2": (rng.standard_normal((128, 512)) * 0.05).astype(np.float32),
        "b2": (rng.standard_normal((128,)) * 0.05).astype(np.float32),
        "W3": (rng.standard_normal((1, 128)) * 0.05).astype(np.float32),
        "b3": (rng.standard_normal((1,)) * 0.05).astype(np.float32),
    }
    out = kernel(**inputs)
    print(out.shape, out.dtype)
